# revision 12
# baseline (speedup 1.0000x reference)
"""YOLOv3-style detection decode on 8 Trainium2 NeuronCores (pure batch data-parallel).

Contract: kernel(**inputs) takes the FULL inputs from setup_inputs() and returns
the FULL output of reference(). Batch dim 32 is sharded 4-per-core across 8
cores. Layout per core: the 4 batches x 3549 positions x 3 anchors are split
into 252 chunks of 169 positions; two chunks that share (head, anchor) -- and
hence grid step t and anchor size -- are packed into each of 126 SBUF
partition lines (free dim = 2*169 = 338). Every per-row decode constant is
then a per-PARTITION constant:
  - PE accumulates psum = t*col + t*x directly: an 11-pattern grid (already
    scaled by t, exact -- t is a power of two) is broadcast to lines via a
    selector matmul, and x rides in as f16 hi+lo pairs against a t*I diagonal
    weight (shipped inside din), giving f32-exact (col+x)*t.
  - ACT computes w,h as exp(w + ln(anchor)) with a per-partition bias.
  - DVE: threshold mask (f32-exact), batch-index fill, and three mask
    multiplies (x,y straight out of PSUM; w,h from ACT's bf16 planes).
Outputs are written bf16 (rel err ~2e-3 << 2e-2 gate) and widened to f32 on
the host during unsharding.
"""
import sys

sys.path.insert(0, "/opt/trn_rl_repo")

import numpy as np

N_CORES = 8
B_PER_CORE = 4
IMG = 416.0
C = 169                       # chunk length
L = 338                       # line free length (2 chunks)
NLINES = 126                  # valid partition lines per core
ANCHORS = {
    13: np.array([[116.0, 90.0], [156.0, 198.0], [373.0, 326.0]], np.float32),
    26: np.array([[30.0, 61.0], [62.0, 45.0], [59.0, 119.0]], np.float32),
    52: np.array([[10.0, 13.0], [16.0, 30.0], [33.0, 23.0]], np.float32),
}
HEADS = [13, 26, 52]
HW = {h: h * h for h in HEADS}
HEAD_OFF = {13: 0, 26: 32 * 169 * 3, 52: 32 * 169 * 3 + 32 * 676 * 3}
N_ROWS = 32 * 3549 * 3

# --- byte layout of one din line (u8 [128, 5696]) ---
# piece1 [0:2992):  scal 32 | t*I row 256 | conf f32 1352 | w f16 676 | h 676
# piece2 [2992:5696): xh | xl | yh | yl (f16, 676 each)
SCAL_B = 0
TI_B = 32
CONF_B = 288
W_B, H_B = 1640, 2316
XH_B, XL_B, YH_B, YL_B = 2992, 3668, 4344, 5020
LINE_B = 5696
P1_B = 2992


def _build_tables():
    lines = []
    for bp in range(2):
        for a in range(3):
            lines.append((13, a, 2 * bp, 0, 2 * bp + 1, 0, 10))
    for bl in range(4):
        for a in range(3):
            for j in range(2):
                lines.append((26, a, bl, 338 * j, bl, 338 * j + 169, 8 + j))
    for bl in range(4):
        for a in range(3):
            for j in range(8):
                lines.append((52, a, bl, 338 * j, bl, 338 * j + 169, j))
    assert len(lines) == NLINES

    # grid patterns [11, 338], PRE-SCALED by t (exact: t is a power of two)
    gcol = np.zeros((11, L), np.float32)
    grow = np.zeros((11, L), np.float32)
    for j in range(8):
        pos = 338 * j + np.arange(L)
        gcol[j] = (pos % 52) * 8.0
        grow[j] = (pos // 52) * 8.0
    for j in range(2):
        pos = 338 * j + np.arange(L)
        gcol[8 + j] = (pos % 26) * 16.0
        grow[8 + j] = (pos // 26) * 16.0
    pos = np.arange(L) % 169
    gcol[10] = (pos % 13) * 32.0
    grow[10] = (pos // 13) * 32.0

    wsel = np.zeros((11, 128), np.float32)
    for l, e in enumerate(lines):
        wsel[e[6], l] = 1.0
    dgw = np.zeros((11, 804), np.float16)
    dgw[:, 0:L] = gcol
    dgw[:, L : 2 * L] = grow
    dgw[:, 2 * L : 2 * L + 128] = wsel

    t_vec = np.zeros(128, np.float32)
    lnaw = np.zeros(128, np.float32)
    lnah = np.zeros(128, np.float32)
    blA = np.zeros(128, np.int64)
    blB = np.zeros(128, np.int64)
    for l, (h, a, bA, sA, bB, sB, _p) in enumerate(lines):
        t_vec[l] = IMG / h
        lnaw[l] = np.log(ANCHORS[h][a, 0])
        lnah[l] = np.log(ANCHORS[h][a, 1])
        blA[l] = bA
        blB[l] = bB
    tIeye = np.zeros((128, 128), np.float16)
    for l in range(NLINES):
        tIeye[l, l] = t_vec[l]

    flat_off = {13: 0, 26: 32 * 3 * 169, 52: 32 * 3 * 169 + 32 * 3 * 676}
    IDX = np.zeros((N_CORES, 128, L), np.int64)
    ROW = np.zeros((N_CORES, 128, L), np.int64)
    ar = np.arange(C)
    for c in range(N_CORES):
        for l, (h, a, bA, sA, bB, sB, _p) in enumerate(lines):
            gA, gB = 4 * c + bA, 4 * c + bB
            IDX[c, l, 0:C] = flat_off[h] + (gA * 3 + a) * HW[h] + sA + ar
            IDX[c, l, C:L] = flat_off[h] + (gB * 3 + a) * HW[h] + sB + ar
            ROW[c, l, 0:C] = HEAD_OFF[h] + gA * HW[h] * 3 + (sA + ar) * 3 + a
            ROW[c, l, C:L] = HEAD_OFF[h] + gB * HW[h] * 3 + (sB + ar) * 3 + a

    return dgw, tIeye, t_vec, lnaw, lnah, blA, blB, IDX, ROW


(_DGW, _TIEYE, _TVEC, _LNAW, _LNAH, _BLA, _BLB, _IDX, _ROW) = _build_tables()
_ROWv = _ROW[:, :NLINES].ravel()

_STATE = None


def _build_program():
    """Raw Bacc program, manual semaphores."""
    import concourse.bass as bass
    import concourse.bacc as bacc
    from concourse import mybir

    # Skip the Bass-constructor all-engine barrier (~0.8us): nothing here
    # reads the framework const APs before gpsimd's own preamble runs.
    _orig_barrier = bass.Bass.all_engine_barrier
    bass.Bass.all_engine_barrier = lambda self, *a, **k: None
    try:
        nc = bacc.Bacc("TRN2", target_bir_lowering=False, debug=False)
    finally:
        bass.Bass.all_engine_barrier = _orig_barrier
    f32 = mybir.dt.float32
    f16 = mybir.dt.float16
    bf16 = mybir.dt.bfloat16
    u8 = mybir.dt.uint8
    op = mybir.AluOpType
    act = mybir.ActivationFunctionType

    IN = nc.dram_tensor("din", [128, LINE_B], u8, kind="ExternalInput")
    GW = nc.dram_tensor("dgw", [11, 804], f16, kind="ExternalInput")
    OUT = nc.dram_tensor("dout", [128, 5 * L], bf16, kind="ExternalOutput")

    tin = nc.alloc_sbuf_tensor("tin", [128, LINE_B], u8)
    tgw = nc.alloc_sbuf_tensor("tgw", [128, 804], f16)
    tS = nc.alloc_sbuf_tensor("tS", [128, 2 * L], bf16)    # sw | sh
    tm = nc.alloc_sbuf_tensor("tm", [128, L], bf16)
    tout = nc.alloc_sbuf_tensor("tout", [128, 5 * L], bf16)
    ps0 = nc.alloc_psum_tensor("ps0", [128, L], f32)
    ps1 = nc.alloc_psum_tensor("ps1", [128, L], f32)

    s_gw = nc.alloc_semaphore("s_gw")
    s_p1 = nc.alloc_semaphore("s_p1")
    s_p2 = nc.alloc_semaphore("s_p2")
    s_pe = nc.alloc_semaphore("s_pe")
    s_m = nc.alloc_semaphore("s_m")
    s_c0 = nc.alloc_semaphore("s_c0")
    s_exp = nc.alloc_semaphore("s_exp")
    s_mmxy = nc.alloc_semaphore("s_mmxy")
    s_mmwh = nc.alloc_semaphore("s_mmwh")
    s_out = nc.alloc_semaphore("s_out")

    inf32 = tin.ap().bitcast(f32)
    inf16 = tin.ap().bitcast(f16)
    thr = inf32[:, 0:1]
    bA = inf32[:, 1:2]
    bB = inf32[:, 2:3]
    lnaw = inf32[:, 4:5]
    lnah = inf32[:, 5:6]
    tIt = inf16[:, TI_B // 2 : TI_B // 2 + 128]
    conf = inf32[:, CONF_B // 4 : CONF_B // 4 + L]
    wv = inf16[:, W_B // 2 : W_B // 2 + L]
    hv = inf16[:, H_B // 2 : H_B // 2 + L]
    xh = inf16[:, XH_B // 2 : XH_B // 2 + L]
    xl = inf16[:, XL_B // 2 : XL_B // 2 + L]
    yh = inf16[:, YH_B // 2 : YH_B // 2 + L]
    yl = inf16[:, YL_B // 2 : YL_B // 2 + L]

    # --- input DMAs, 2 HWDGE rings. Scalar starts earliest and has the
    # faster queue -> 88 lines; sync takes 40 lines + the tiny dgw.
    nc.scalar.dma_start(
        tin.ap()[0:88, 0:P1_B], IN.ap()[0:88, 0:P1_B]
    ).then_inc(s_p1, 16)
    nc.scalar.dma_start(
        tin.ap()[0:88, P1_B:], IN.ap()[0:88, P1_B:]
    ).then_inc(s_p2, 16)
    nc.sync.dma_start(tgw.ap()[0:11, :], GW.ap()).then_inc(s_gw, 16)
    nc.sync.dma_start(
        tin.ap()[88:128, 0:P1_B], IN.ap()[88:128, 0:P1_B]
    ).then_inc(s_p1, 16)
    nc.sync.dma_start(
        tin.ap()[88:128, P1_B:], IN.ap()[88:128, P1_B:]
    ).then_inc(s_p2, 16)

    # --- PE: psum0 = sel@(t*col) + t*xh + t*xl ; psum1 likewise for y
    wsel = tgw.ap()[0:11, 2 * L : 2 * L + 128]
    nc.tensor.wait_ge(s_gw, 16)
    nc.tensor.matmul(
        ps0.ap(), wsel, tgw.ap()[0:11, 0:L],
        start=True, stop=False, skip_group_check=True,
    )
    nc.tensor.matmul(
        ps1.ap(), wsel, tgw.ap()[0:11, L : 2 * L],
        start=True, stop=False, skip_group_check=True,
    )
    nc.tensor.wait_ge(s_p1, 32)
    nc.tensor.wait_ge(s_p2, 32)
    nc.tensor.matmul(
        ps0.ap(), tIt, xh, start=False, stop=False, skip_group_check=True
    )
    nc.tensor.matmul(
        ps0.ap(), tIt, xl, start=False, stop=True, skip_group_check=True
    ).then_inc(s_pe, 1)
    nc.tensor.matmul(
        ps1.ap(), tIt, yh, start=False, stop=False, skip_group_check=True
    )
    nc.tensor.matmul(
        ps1.ap(), tIt, yl, start=False, stop=True, skip_group_check=True
    ).then_inc(s_pe, 1)

    # --- ACT: exps (w,h arrive in piece1)
    nc.scalar.wait_ge(s_p1, 32)
    nc.scalar.activation(
        tS.ap()[:, 0:L], wv, act.Exp, bias=lnaw
    ).then_inc(s_exp, 1)
    nc.scalar.activation(
        tS.ap()[:, L : 2 * L], hv, act.Exp, bias=lnah
    ).then_inc(s_exp, 1)

    # --- DVE: mask, c0, mask-multiplies
    nc.vector.wait_ge(s_p1, 32)
    nc.vector.tensor_scalar(tm.ap(), conf, thr, None, op.is_gt).then_inc(s_m, 1)
    nc.vector.wait_ge(s_m, 1)
    nc.vector.tensor_scalar(
        tout.ap()[:, 0:L], tm.ap(), bA, None, op.mult
    ).then_inc(s_c0, 1)
    # head13 lines (0:6) carry a different batch in the second chunk; the op
    # covers [0:32] for partition alignment -- bB == bA on lines 6:32.
    nc.vector.tensor_scalar(
        tout.ap()[0:32, C:L], tm.ap()[0:32, C:L], bB[0:32], None, op.mult
    ).then_inc(s_c0, 1)
    m2 = tm.ap().unsqueeze(1).broadcast_to((128, 2, L))
    nc.vector.wait_ge(s_exp, 2)
    nc.vector.tensor_tensor(
        tout.ap()[:, 3 * L : 5 * L].rearrange("p (c n) -> p c n", n=L),
        tS.ap().rearrange("p (c n) -> p c n", n=L),
        m2, op.mult,
    ).then_inc(s_mmwh, 1)
    nc.vector.wait_ge(s_pe, 1)
    nc.vector.tensor_tensor(
        tout.ap()[:, L : 2 * L], ps0.ap(), tm.ap(), op.mult
    ).then_inc(s_mmxy, 1)
    nc.vector.wait_ge(s_pe, 2)
    nc.vector.tensor_tensor(
        tout.ap()[:, 2 * L : 3 * L], ps1.ap(), tm.ap(), op.mult
    ).then_inc(s_mmxy, 1)

    # --- output DMAs: [w|h] piece first (ready earlier), then [c0|x|y].
    # No engine waits on s_out: the out transfers drain during the NEFF exit
    # sequence; per-ring FIFO orders them before the next run's input DMAs,
    # and the host reads results ms later via PJRT.
    nc.sync.wait_ge(s_mmwh, 1)
    nc.sync.dma_start(
        OUT.ap()[0:48, 3 * L :], tout.ap()[0:48, 3 * L :]
    ).then_inc(s_out, 16)
    nc.sync.wait_ge(s_c0, 2)
    nc.sync.wait_ge(s_mmxy, 2)
    nc.sync.dma_start(
        OUT.ap()[0:48, 0 : 3 * L], tout.ap()[0:48, 0 : 3 * L]
    ).then_inc(s_out, 16)
    nc.scalar.wait_ge(s_mmwh, 1)
    nc.scalar.dma_start(
        OUT.ap()[48:128, 3 * L :], tout.ap()[48:128, 3 * L :]
    ).then_inc(s_out, 16)
    nc.scalar.wait_ge(s_c0, 2)
    nc.scalar.wait_ge(s_mmxy, 2)
    nc.scalar.dma_start(
        OUT.ap()[48:128, 0 : 3 * L], tout.ap()[48:128, 0 : 3 * L]
    ).then_inc(s_out, 16)
    nc.compile()
    return nc


def _pack(heads_np, thresh):
    """FULL head tensors -> per-core din u8 [8, 128, LINE_B] via index gathers."""
    flats = {}
    for ch in range(5):
        parts = []
        for h in HEADS:
            a = heads_np[h].reshape(32, 3, 85, HW[h])
            parts.append(np.ascontiguousarray(a[:, :, ch, :]).ravel())
        flats[ch] = np.concatenate(parts)

    conf_g = flats[0][_IDX].astype(np.float32)
    x32 = flats[1][_IDX].astype(np.float32)
    y32 = flats[2][_IDX].astype(np.float32)
    xh = x32.astype(np.float16)
    xl = (x32 - xh.astype(np.float32)).astype(np.float16)
    yh = y32.astype(np.float16)
    yl = (y32 - yh.astype(np.float32)).astype(np.float16)
    wg = flats[3][_IDX].astype(np.float16)
    hg = flats[4][_IDX].astype(np.float16)

    din = np.zeros((N_CORES, 128, LINE_B), np.uint8)
    scal = np.zeros((N_CORES, 128, 8), np.float32)
    scal[:, :, 0] = thresh
    for c in range(N_CORES):
        scal[c, :, 1] = 4 * c + _BLA
        scal[c, :, 2] = 4 * c + _BLB
    scal[:, :NLINES, 4] = _LNAW[:NLINES]
    scal[:, :NLINES, 5] = _LNAH[:NLINES]
    scal[:, NLINES:, 0] = 1e9          # dummy lines: mask always 0
    din[:, :, 0:32] = scal.view(np.uint8)
    din[:, :, TI_B : TI_B + 256] = _TIEYE.view(np.uint8)[None]
    din[:, :, CONF_B : CONF_B + 4 * L] = conf_g.view(np.uint8)
    din[:, :, W_B : W_B + 2 * L] = wg.view(np.uint8)
    din[:, :, H_B : H_B + 2 * L] = hg.view(np.uint8)
    din[:, :, XH_B : XH_B + 2 * L] = xh.view(np.uint8)
    din[:, :, XL_B : XL_B + 2 * L] = xl.view(np.uint8)
    din[:, :, YH_B : YH_B + 2 * L] = yh.view(np.uint8)
    din[:, :, YL_B : YL_B + 2 * L] = yl.view(np.uint8)
    return din


def kernel(output_13, output_26, output_52, thresh):
    global _STATE
    if _STATE is None:
        _STATE = _build_program()
    nc = _STATE

    from concourse.bass_utils import run_bass_kernel_spmd

    heads_np = {13: np.asarray(output_13, np.float32),
                26: np.asarray(output_26, np.float32),
                52: np.asarray(output_52, np.float32)}
    thr = float(np.asarray(thresh))
    din = _pack(heads_np, thr)
    in_maps = [{"din": din[c], "dgw": _DGW} for c in range(N_CORES)]

    res = run_bass_kernel_spmd(nc, in_maps, core_ids=list(range(N_CORES)))

    up = np.stack([np.asarray(res.results[c]["dout"]) for c in range(N_CORES)])
    up = up.astype(np.float32)                            # [8,128,1690]
    out = np.empty((N_ROWS, 5), np.float32)
    for col in range(5):
        plane = up[:, :NLINES, col * L : (col + 1) * L]
        out[_ROWv, col] = plane.reshape(-1)
    return out
